# revision 62
# baseline (speedup 1.0000x reference)
"""Quantized int8 matmul on 8 TRN2 NeuronCores.

Math: out = ((x - ZP_X) * SCALE_X) @ ((y - ZP_Y) * SCALE_Y)
Implemented as: out = [(x - ZP_X) @ (y - ZP_Y)] * (SCALE_X * SCALE_Y)
The zero-point-shifted values (range ~[-150, 155]) are rounded to
fp8e4 (e4m3, RNE): per-element relative rounding error ~3% which
averages out over the K=4096 contraction to ~1e-2 output rel err,
within the 2e-2 gate. e4m3 x e4m3 products are exact in the PE's
e10m10 internal format; accumulation is fp32 PSUM. fp8 enables
MatmulPerfMode.DoubleRow: 2 fp8 weights per PE cell -> each matmul
contracts 256 k-values (2 x 128 subtiles) at ~1 col/cycle.

Sharding: 2D grid, M split 4 ways x N split 2 ways. Per core:
x shard [1024, 4096] (stored transposed [K, m_loc]), y shard
[4096, 2048], out [1024, 2048]. No collectives; host shards/gathers.

Per-core schedule: the whole y shard is DMA'd in 16 wide tiles
[128, 2, 2048] (2KB DRAM chunks), converted once to fp8 and kept
RESIDENT in SBUF (64KB/partition), stored with each (k-pair, n-block)
group's pair-rows adjacent so the DoubleRow moving operand reads
1024 contiguous bytes/partition (strided pair-rows cost +20%/MM).
The 8 output sub-passes (4 n-blocks x 2 m-quads, 4 PSUM banks each,
ping-pong) then stream from SBUF. Phase A = both m-quads of block 0,
interleaved per k-pair to chase the y stream (depends only on the
fast DVE block-0 converts); the rest free-run with no DMA dependence.

Hardware lessons baked in:
 - DVE elementwise ops with >1024-elem inner dim run ~8x below rate:
   all converts are [128, 2, 512] chunks.
 - gpsimd tensor ops are software (Q7) at ~7ns/elem: never convert
   there.
 - Early DMA completion semaphores trail data by 2-6us (SDMA engines
   round-robin all queued work): one input ring in FIFO order + a
   long PE warm-up covers the latency; the first matmul's exact
   inputs (y0 block-0/1 half) lead the ring. Fewer/bigger DMAs win:
   x loads ride 8 512KB quad DMAs, each sub-pass stores once (1MB).
 - The chip toggles 2.4 <-> ~2.0 GHz (P0 power state) under sustained
   load; MM 216 vs 259 ns. The schedule shows zero PE gaps at 2.0GHz
   and ~4us total at 2.4GHz (data-arrival-bound early phase).

Engine split per core:
  PE     - 20 warm-up dummies + 512 DoubleRow matmuls (256k x 128m x 512n)
  SP     - HWDGE ring: all x/y input loads + all non-final out stores,
           in consumption order
  ACT    - y converts for blocks 1-3; half the final eviction + its
           store on the ACT HWDGE ring
  DVE    - x converts (one [P,2,1024] per k-step), y block-0 converts,
           PSUM evictions
  GpSimd - warm-up memsets only
"""

import numpy as np

SCALE_X, ZP_X = 0.0215, -25
SCALE_Y, ZP_Y = 0.0176, 18
M, K, N = 4096, 4096, 4096
N_CORES = 8
M_SH, N_SH = 4, 2  # core grid: M split x N split
P = 128
NBLK = 512  # matmul moving free dim = one PSUM bank of fp32
N_WARM = 20  # PE warm-up dummy matmuls (covers early DMA-completion latency)


def build_nc(m_loc, k, n_loc):
    from contextlib import ExitStack

    import concourse.mybir as mybir
    import concourse.tile as tile
    from concourse import bacc
    from concourse.bass import ds, ts

    fp32 = mybir.dt.float32
    fp8 = mybir.dt.float8e4
    int8 = mybir.dt.int8
    Copy = mybir.ActivationFunctionType.Copy
    DoubleRow = mybir.MatmulPerfMode.DoubleRow

    MT = m_loc // P  # m tiles (8) = 2 quads of 4
    KT = k // P  # contraction tiles (32)
    Q = KT // 2  # DoubleRow k-pairs (16)
    NB = n_loc // NBLK  # output column blocks (4)
    MQ = MT // 4  # m quads (2)
    nsubs = NB * MQ  # 8 sub-passes of 4 PSUM banks each

    nc = bacc.Bacc(None, debug=False)
    xt = nc.declare_dram_parameter("xt", [k, m_loc], int8, isOutput=False)
    y = nc.declare_dram_parameter("y", [k, n_loc], int8, isOutput=False)
    out = nc.declare_dram_parameter("out", [m_loc, n_loc], fp32, isOutput=True)

    # DRAM views grouping k as (batch, tile-in-batch, partition)
    xt_r4 = xt.rearrange("(g b p) m -> g p b m", b=4, p=P)
    y_r = y.rearrange("(q b p) n -> q p b n", b=2, p=P)
    # out as [partition, m-tile, n]: lets one DMA store a whole 4-psum
    # sub-pass (fewer completion semaphores to process at the end)
    out_r = out.rearrange("(t p) n -> p t n", p=P)

    with ExitStack() as ctx:
        tc = ctx.enter_context(tile.TileContext(nc))
        wm_pool = ctx.enter_context(tc.tile_pool(name="wm", bufs=1))
        xi_pool = ctx.enter_context(tc.tile_pool(name="xi", bufs=4))
        xt_pool = ctx.enter_context(tc.tile_pool(name="xtb", bufs=1, side="right"))
        yi_pool = ctx.enter_context(tc.tile_pool(name="yi", bufs=13))
        yb_pool = ctx.enter_context(tc.tile_pool(name="yb", bufs=1, side="right"))
        ob_pool = ctx.enter_context(tc.tile_pool(name="ob", bufs=3))
        ps_pool = ctx.enter_context(tc.tile_pool(name="ps", bufs=8, space="PSUM"))

        # PE warm-up: dummy matmuls on zeroed tiles during the startup DMA
        # window so the HAM clock-gate opens before the real stream begins.
        wm_w = wm_pool.tile([P, P], fp8)
        wm_s = wm_pool.tile([P, NBLK], fp8)
        nc.gpsimd.memset(wm_w[:], 0.0)
        nc.gpsimd.memset(wm_s[:], 0.0)
        ps_warm = ps_pool.tile([P, NBLK], fp32, tag="ps", name="warm")
        for _ in range(N_WARM):
            nc.tensor.matmul(ps_warm[:], wm_w[:], wm_s[:], start=True, stop=True)

        # Persistent fp8 operands: x^T [p, kt, m]; y [p, q, 2*bi+pair, n-blk]
        # (pair-rows of each (q, block) group adjacent, so the DoubleRow
        # moving operand reads 1024 contiguous bytes per partition)
        xT = xt_pool.tile([P, KT, m_loc], fp8)
        yB = yb_pool.tile([P, Q, 2 * NB, NBLK], fp8)

        x_tiles = {}

        def emit_xq_dma(g):
            # one 512KB DMA covers k-pairs 2g and 2g+1
            if g >= Q // 2:
                return
            xi = xi_pool.tile([P, 4, m_loc], int8, name=f"xi_{g}", tag="xi")
            nc.sync.dma_start(xi[:], xt_r4[g])
            x_tiles[g] = xi

        def emit_x_convert(g, h):
            # one [P, 2, 1024] DVE convert per k-step: keeps the per-step
            # DVE load under the 1.73us phase-A budget (DVE's slow path
            # also triggers on big fused elementwise ops)
            if g >= Q // 2:
                return
            nc.vector.tensor_scalar_add(
                xT[:, ds(4 * g + 2 * h, 2), :],
                x_tiles[g][:, ds(2 * h, 2), :],
                float(-ZP_X),
            )

        def emit_y(q):
            yi = yi_pool.tile([P, 2, n_loc], int8, name=f"yi_{q}", tag="yi")
            # All input DMAs ride one ring (SP) in consumption order --
            # completion semaphores trail data by microseconds and are
            # processed roughly in order, so ring order = consumption
            # order minimizes critical-path completion latency. q=0 is
            # split so its phase-A-critical half completes first.
            half = 2 * NBLK
            if q == 0:
                nc.sync.dma_start(
                    yi[:, :, ds(0, half)], y_r[q, :, :, ds(0, half)]
                )
                nc.sync.dma_start(
                    yi[:, :, ds(half, half)], y_r[q, :, :, ds(half, half)]
                )
            elif q < 3:
                # ride the otherwise-idle ACT ring during the completion
                # ramp: y1/y2 complete in parallel with the SP ring's
                # y0 + x quad instead of queueing behind them
                nc.scalar.dma_start(yi[:], y_r[q])
            else:
                nc.sync.dma_start(yi[:], y_r[q])
            # DVE runs wide (2048-inner) elementwise ops ~8x below rate;
            # convert per (q, block): [P, 2, 512], inner 512. Block 0 feeds
            # phase A -> always DVE (fast, tight deadline); blocks 1-3 have
            # relaxed deadlines -> ACT FIFO churns through them.
            for bi in range(NB):
                src = yi[:, :, ds(bi * NBLK, NBLK)]
                dst = yB[:, q, ds(2 * bi, 2), :]
                if bi == 0:
                    nc.vector.tensor_scalar_add(dst, src, float(-ZP_Y))
                else:
                    nc.scalar.activation(dst, src, Copy, bias=float(-ZP_Y))

        # Sub-pass order: phase A = both m-quads of block 0, chasing the y
        # stream -- phase A depends ONLY on the fast DVE block-0 converts.
        # Blocks 1-3 follow with progressively relaxed convert deadlines
        # that the ACT FIFO (3 chunks/q) meets comfortably.
        subs = [(0, 0), (0, 1), (1, 0), (1, 1), (2, 0), (2, 1), (3, 0), (3, 1)]
        assert len(subs) == nsubs

        def emit_sub_q(s, q, psums, mis=(0, 1, 2, 3)):
            bi, mq = subs[s]
            for mi in mis:
                mt = mq * 4 + mi
                nc.tensor.matmul(
                    psums[mi][:],
                    xT[:, ds(2 * q, 2), ts(mt, P)],
                    yB[:, q, ds(2 * bi, 2), :],
                    start=(q == 0),
                    stop=(q == Q - 1),
                    perf_mode=DoubleRow,
                )

        scale = float(SCALE_X * SCALE_Y)

        def evict_sub(s, psums):
            # Evictions on DVE (ACT's FIFO is full of y converts until
            # ~60us); one batched 1MB store per sub-pass on the SP ring
            # (completion-semaphore processing, not data movement, is the
            # scheduling tax). The final sub-pass evicts on DVE+ACT in
            # parallel and drains on both HWDGE rings.
            bi, mq = subs[s]
            col = bi * NBLK
            ob = ob_pool.tile([P, 4, NBLK], fp32, name=f"ob_{s}", tag="ob")
            last = s == nsubs - 1
            for mi in range(4):
                if last and mi >= 2:
                    nc.scalar.activation(
                        ob[:, mi, :], psums[mi][:], Copy, scale=scale
                    )
                else:
                    nc.vector.tensor_scalar_mul(ob[:, mi, :], psums[mi][:], scale)
            if last:
                nc.sync.dma_start(
                    out_r[:, ds(mq * 4, 2), ds(col, NBLK)], ob[:, ds(0, 2), :]
                )
                nc.scalar.dma_start(
                    out_r[:, ds(mq * 4 + 2, 2), ds(col, NBLK)], ob[:, ds(2, 2), :]
                )
            else:
                nc.sync.dma_start(out_r[:, ds(mq * 4, 4), ds(col, NBLK)], ob[:])

        def alloc_psums(s):
            return [
                ps_pool.tile([P, NBLK], fp32, tag="ps", name=f"acc_{s}_{i}")
                for i in range(4)
            ]

        # Phase A: y/x streams + sub-passes 0 and 1 interleaved per k-pair.
        # y0 first: its block-0/1 half + convert lead the SP ring / DVE
        # queue (they gate the first real matmul together with x quad 0).
        ps0 = alloc_psums(0)
        ps1 = alloc_psums(1)
        # ring order = first-matmul critical path: y0's block-0/1 half,
        # then x quad 0 (its convert gates the first matmul too), then
        # y0's remaining blocks (ACT converts have slack). Starting the
        # real matmuls EARLIER than this measures worse: phase A then
        # outruns the y completion stream and stalls mid-flight instead.
        yi0 = yi_pool.tile([P, 2, n_loc], int8, name="yi_0", tag="yi")
        half = 2 * NBLK
        nc.sync.dma_start(yi0[:, :, ds(0, half)], y_r[0, :, :, ds(0, half)])
        emit_xq_dma(0)
        nc.sync.dma_start(
            yi0[:, :, ds(half, half)], y_r[0, :, :, ds(half, half)]
        )
        for bi in range(NB):
            src = yi0[:, :, ds(bi * NBLK, NBLK)]
            dst = yB[:, 0, ds(2 * bi, 2), :]
            if bi == 0:
                nc.vector.tensor_scalar_add(dst, src, float(-ZP_Y))
            else:
                nc.scalar.activation(dst, src, Copy, bias=float(-ZP_Y))
        emit_x_convert(0, 0)
        emit_x_convert(0, 1)
        for q in range(Q):
            if q > 0:
                emit_y(q)
            if q % 2 == 0:
                emit_xq_dma(q // 2 + 1)
            # convert the x half needed two k-steps ahead, one per step
            emit_x_convert((q + 2) // 2, (q + 2) % 2)
            emit_sub_q(0, q, ps0)
            emit_sub_q(1, q, ps1)
        evict_sub(0, ps0)
        evict_sub(1, ps1)

        # Phase B: sub-passes 2..7 free-run from SBUF
        for s in range(2, nsubs):
            psums = alloc_psums(s)
            for q in range(Q):
                emit_sub_q(s, q, psums)
            evict_sub(s, psums)

    nc.compile()
    return nc


_NC_CACHE = None
LAST_RESULT = None  # BassKernelResults of the most recent run (for profiling)


def _ensure_ntff_hook():
    """concourse's trace path imports antenv.axon_hooks, which is absent
    from this container's antenv stub. Provide it (with the real libaxon
    ctypes hook when available) so tracing works -- or degrades cleanly."""
    import sys
    import types

    try:
        import antenv.axon_hooks  # noqa: F401

        return
    except ImportError:
        pass
    mod = types.ModuleType("antenv.axon_hooks")
    holder = [None]
    mod.set_axon_ntff_profile_hook = lambda h: holder.__setitem__(0, h)
    mod.get_axon_ntff_profile_hook = lambda: holder[0]
    sys.modules["antenv.axon_hooks"] = mod
    try:
        import antenv

        antenv.axon_hooks = mod
    except ImportError:
        pass
    try:
        from trn_agent_boot.trn_boot import _ntff_profile_via_ctypes

        mod.set_axon_ntff_profile_hook(
            _ntff_profile_via_ctypes("/opt/axon/libaxon_pjrt.so")
        )
    except Exception:
        pass  # no hook -> concourse logs a warning and skips tracing


def kernel(x, y):
    global _NC_CACHE, LAST_RESULT
    _ensure_ntff_hook()
    from concourse.bass_utils import run_bass_kernel_spmd

    x = np.asarray(x)
    y = np.asarray(y)
    assert x.shape == (M, K) and y.shape == (K, N), (x.shape, y.shape)
    x8 = x.astype(np.int8) if x.dtype != np.int8 else x
    y8 = y.astype(np.int8) if y.dtype != np.int8 else y

    m_loc = M // M_SH
    n_loc = N // N_SH
    if _NC_CACHE is None:
        _NC_CACHE = build_nc(m_loc, K, n_loc)
    nc = _NC_CACHE

    in_maps = []
    for c in range(N_CORES):
        mi, nj = divmod(c, N_SH)
        in_maps.append(
            {
                "xt": np.ascontiguousarray(x8[mi * m_loc : (mi + 1) * m_loc].T),
                "y": np.ascontiguousarray(y8[:, nj * n_loc : (nj + 1) * n_loc]),
            }
        )
    res = run_bass_kernel_spmd(nc, in_maps, core_ids=list(range(N_CORES)))
    LAST_RESULT = res
    full = np.empty((M, N), dtype=np.float32)
    for c in range(N_CORES):
        mi, nj = divmod(c, N_SH)
        full[mi * m_loc : (mi + 1) * m_loc, nj * n_loc : (nj + 1) * n_loc] = (
            np.asarray(res.results[c]["out"])
        )
    return full


# revision 63
# speedup vs baseline: 1.1284x; 1.1284x over previous
"""Quantized int8 matmul on 8 TRN2 NeuronCores.

Math: out = ((x - ZP_X) * SCALE_X) @ ((y - ZP_Y) * SCALE_Y)
Implemented as: out = [(x - ZP_X) @ (y - ZP_Y)] * (SCALE_X * SCALE_Y)
The zero-point-shifted values (range ~[-150, 155]) are rounded to
fp8e4 (e4m3, RNE): per-element relative rounding error ~3% which
averages out over the K=4096 contraction to ~1e-2 output rel err,
within the 2e-2 gate. e4m3 x e4m3 products are exact in the PE's
e10m10 internal format; accumulation is fp32 PSUM. fp8 enables
MatmulPerfMode.DoubleRow: 2 fp8 weights per PE cell -> each matmul
contracts 256 k-values (2 x 128 subtiles) at ~1 col/cycle.

Sharding: 2D grid, M split 4 ways x N split 2 ways. Per core:
x shard [1024, 4096] (stored transposed [K, m_loc]), y shard
[4096, 2048], out [1024, 2048]. No collectives; host shards/gathers.

Per-core schedule: the whole y shard is DMA'd in 16 wide tiles
[128, 2, 2048] (2KB DRAM chunks), converted once to fp8 and kept
RESIDENT in SBUF (64KB/partition), stored with each (k-pair, n-block)
group's pair-rows adjacent so the DoubleRow moving operand reads
1024 contiguous bytes/partition (strided pair-rows cost +20%/MM).
The 8 output sub-passes (4 n-blocks x 2 m-quads, 4 PSUM banks each,
ping-pong) then stream from SBUF. Phase A = both m-quads of block 0,
interleaved per k-pair to chase the y stream (depends only on the
fast DVE block-0 converts); the rest free-run with no DMA dependence.

Hardware lessons baked in:
 - DVE elementwise ops with >1024-elem inner dim run ~8x below rate:
   all converts are [128, 2, 512] chunks.
 - gpsimd tensor ops are software (Q7) at ~7ns/elem: never convert
   there.
 - Early DMA completion semaphores trail data by 2-6us (SDMA engines
   round-robin all queued work): one input ring in FIFO order + a
   long PE warm-up covers the latency; the first matmul's exact
   inputs (y0 block-0/1 half) lead the ring. Fewer/bigger DMAs win:
   x loads ride 8 512KB quad DMAs, each sub-pass stores once (1MB).
 - The chip toggles 2.4 <-> ~2.0 GHz (P0 power state) under sustained
   load; MM 216 vs 259 ns. The schedule shows zero PE gaps at 2.0GHz
   and ~4us total at 2.4GHz (data-arrival-bound early phase).

Engine split per core:
  PE     - 20 warm-up dummies + 512 DoubleRow matmuls (256k x 128m x 512n)
  SP     - HWDGE ring: all x/y input loads + all non-final out stores,
           in consumption order
  ACT    - y converts for blocks 1-3; half the final eviction + its
           store on the ACT HWDGE ring
  DVE    - x converts (one [P,2,1024] per k-step), y block-0 converts,
           PSUM evictions
  GpSimd - warm-up memsets only
"""

import numpy as np

SCALE_X, ZP_X = 0.0215, -25
SCALE_Y, ZP_Y = 0.0176, 18
M, K, N = 4096, 4096, 4096
N_CORES = 8
M_SH, N_SH = 4, 2  # core grid: M split x N split
P = 128
NBLK = 512  # matmul moving free dim = one PSUM bank of fp32
N_WARM = 20  # PE warm-up dummy matmuls (covers early DMA-completion latency)


def build_nc(m_loc, k, n_loc):
    from contextlib import ExitStack

    import concourse.mybir as mybir
    import concourse.tile as tile
    from concourse import bacc
    from concourse.bass import ds, ts

    fp32 = mybir.dt.float32
    fp8 = mybir.dt.float8e4
    int8 = mybir.dt.int8
    Copy = mybir.ActivationFunctionType.Copy
    DoubleRow = mybir.MatmulPerfMode.DoubleRow

    MT = m_loc // P  # m tiles (8) = 2 quads of 4
    KT = k // P  # contraction tiles (32)
    Q = KT // 2  # DoubleRow k-pairs (16)
    NB = n_loc // NBLK  # output column blocks (4)
    MQ = MT // 4  # m quads (2)
    nsubs = NB * MQ  # 8 sub-passes of 4 PSUM banks each

    nc = bacc.Bacc(None, debug=False)
    xt = nc.declare_dram_parameter("xt", [k, m_loc], int8, isOutput=False)
    y = nc.declare_dram_parameter("y", [k, n_loc], int8, isOutput=False)
    out = nc.declare_dram_parameter("out", [m_loc, n_loc], fp32, isOutput=True)

    # DRAM views grouping k as (batch, tile-in-batch, partition)
    xt_r4 = xt.rearrange("(g b p) m -> g p b m", b=4, p=P)
    y_r = y.rearrange("(q b p) n -> q p b n", b=2, p=P)
    # out as [partition, m-tile, n]: lets one DMA store a whole 4-psum
    # sub-pass (fewer completion semaphores to process at the end)
    out_r = out.rearrange("(t p) n -> p t n", p=P)

    with ExitStack() as ctx:
        tc = ctx.enter_context(tile.TileContext(nc))
        wm_pool = ctx.enter_context(tc.tile_pool(name="wm", bufs=1))
        xi_pool = ctx.enter_context(tc.tile_pool(name="xi", bufs=4))
        xt_pool = ctx.enter_context(tc.tile_pool(name="xtb", bufs=1, side="right"))
        yi_pool = ctx.enter_context(tc.tile_pool(name="yi", bufs=13))
        yb_pool = ctx.enter_context(tc.tile_pool(name="yb", bufs=1, side="right"))
        ob_pool = ctx.enter_context(tc.tile_pool(name="ob", bufs=3))
        ps_pool = ctx.enter_context(tc.tile_pool(name="ps", bufs=8, space="PSUM"))

        # PE warm-up: dummy matmuls on zeroed tiles during the startup DMA
        # window so the HAM clock-gate opens before the real stream begins.
        wm_w = wm_pool.tile([P, P], fp8)
        wm_s = wm_pool.tile([P, NBLK], fp8)
        nc.gpsimd.memset(wm_w[:], 0.0)
        nc.gpsimd.memset(wm_s[:], 0.0)
        ps_warm = ps_pool.tile([P, NBLK], fp32, tag="ps", name="warm")
        for _ in range(N_WARM):
            nc.tensor.matmul(ps_warm[:], wm_w[:], wm_s[:], start=True, stop=True)

        # Persistent fp8 operands: x^T [p, kt, m]; y [p, q, 2*bi+pair, n-blk]
        # (pair-rows of each (q, block) group adjacent, so the DoubleRow
        # moving operand reads 1024 contiguous bytes per partition)
        xT = xt_pool.tile([P, KT, m_loc], fp8)
        yB = yb_pool.tile([P, Q, 2 * NB, NBLK], fp8)

        x_tiles = {}

        def emit_xq_dma(g):
            # one 512KB DMA covers k-pairs 2g and 2g+1
            if g >= Q // 2:
                return
            xi = xi_pool.tile([P, 4, m_loc], int8, name=f"xi_{g}", tag="xi")
            nc.sync.dma_start(xi[:], xt_r4[g])
            x_tiles[g] = xi

        def emit_x_convert(g, h):
            # one [P, 2, 1024] DVE convert per k-step: keeps the per-step
            # DVE load under the 1.73us phase-A budget (DVE's slow path
            # also triggers on big fused elementwise ops)
            if g >= Q // 2:
                return
            nc.vector.tensor_scalar_add(
                xT[:, ds(4 * g + 2 * h, 2), :],
                x_tiles[g][:, ds(2 * h, 2), :],
                float(-ZP_X),
            )

        def emit_y(q):
            yi = yi_pool.tile([P, 2, n_loc], int8, name=f"yi_{q}", tag="yi")
            # All input DMAs ride one ring (SP) in consumption order --
            # completion semaphores trail data by microseconds and are
            # processed roughly in order, so ring order = consumption
            # order minimizes critical-path completion latency. q=0 is
            # split so its phase-A-critical half completes first.
            half = 2 * NBLK
            if q == 0:
                nc.sync.dma_start(
                    yi[:, :, ds(0, half)], y_r[q, :, :, ds(0, half)]
                )
                nc.sync.dma_start(
                    yi[:, :, ds(half, half)], y_r[q, :, :, ds(half, half)]
                )
            elif q < 3:
                # ride the otherwise-idle ACT ring during the completion
                # ramp: y1/y2 complete in parallel with the SP ring's
                # y0 + x quad instead of queueing behind them
                nc.scalar.dma_start(yi[:], y_r[q])
            else:
                nc.sync.dma_start(yi[:], y_r[q])
            # DVE runs wide (2048-inner) elementwise ops ~8x below rate;
            # convert per (q, block): [P, 2, 512], inner 512. Block 0 feeds
            # phase A -> always DVE (fast, tight deadline); blocks 1-3 have
            # relaxed deadlines -> ACT FIFO churns through them.
            for bi in range(NB):
                src = yi[:, :, ds(bi * NBLK, NBLK)]
                dst = yB[:, q, ds(2 * bi, 2), :]
                if bi == 0:
                    nc.vector.tensor_scalar_add(dst, src, float(-ZP_Y))
                else:
                    nc.scalar.activation(dst, src, Copy, bias=float(-ZP_Y))

        # Sub-pass order: phase A = both m-quads of block 0, chasing the y
        # stream -- phase A depends ONLY on the fast DVE block-0 converts.
        # Blocks 1-3 follow with progressively relaxed convert deadlines
        # that the ACT FIFO (3 chunks/q) meets comfortably.
        subs = [(0, 0), (0, 1), (1, 0), (1, 1), (2, 0), (2, 1), (3, 0), (3, 1)]
        assert len(subs) == nsubs

        def emit_sub_q(s, q, psums, mis=(0, 1, 2, 3)):
            bi, mq = subs[s]
            for mi in mis:
                mt = mq * 4 + mi
                nc.tensor.matmul(
                    psums[mi][:],
                    xT[:, ds(2 * q, 2), ts(mt, P)],
                    yB[:, q, ds(2 * bi, 2), :],
                    start=(q == 0),
                    stop=(q == Q - 1),
                    perf_mode=DoubleRow,
                )

        scale = float(SCALE_X * SCALE_Y)

        def evict_sub(s, psums):
            # Evictions on DVE (ACT's FIFO is full of y converts until
            # ~60us); one batched 1MB store per sub-pass on the SP ring
            # (completion-semaphore processing, not data movement, is the
            # scheduling tax). The final sub-pass evicts on DVE+ACT in
            # parallel and drains on both HWDGE rings.
            bi, mq = subs[s]
            col = bi * NBLK
            ob = ob_pool.tile([P, 4, NBLK], fp32, name=f"ob_{s}", tag="ob")
            last = s == nsubs - 1
            for mi in range(4):
                if last and mi >= 2:
                    nc.scalar.activation(
                        ob[:, mi, :], psums[mi][:], Copy, scale=scale
                    )
                else:
                    nc.vector.tensor_scalar_mul(ob[:, mi, :], psums[mi][:], scale)
            if last:
                nc.sync.dma_start(
                    out_r[:, ds(mq * 4, 2), ds(col, NBLK)], ob[:, ds(0, 2), :]
                )
                nc.scalar.dma_start(
                    out_r[:, ds(mq * 4 + 2, 2), ds(col, NBLK)], ob[:, ds(2, 2), :]
                )
            else:
                nc.sync.dma_start(out_r[:, ds(mq * 4, 4), ds(col, NBLK)], ob[:])

        def alloc_psums(s):
            return [
                ps_pool.tile([P, NBLK], fp32, tag="ps", name=f"acc_{s}_{i}")
                for i in range(4)
            ]

        # Phase A: y/x streams + sub-passes 0 and 1 interleaved per k-pair.
        # y0 first: its block-0/1 half + convert lead the SP ring / DVE
        # queue (they gate the first real matmul together with x quad 0).
        ps0 = alloc_psums(0)
        ps1 = alloc_psums(1)
        # ring order = first-matmul critical path: y0's block-0/1 half,
        # then x quad 0 (its convert gates the first matmul too), then
        # y0's remaining blocks (ACT converts have slack). Starting the
        # real matmuls EARLIER than this measures worse: phase A then
        # outruns the y completion stream and stalls mid-flight instead.
        yi0 = yi_pool.tile([P, 2, n_loc], int8, name="yi_0", tag="yi")
        half = 2 * NBLK
        nc.sync.dma_start(yi0[:, :, ds(0, half)], y_r[0, :, :, ds(0, half)])
        emit_xq_dma(0)
        nc.sync.dma_start(
            yi0[:, :, ds(half, half)], y_r[0, :, :, ds(half, half)]
        )
        for bi in range(NB):
            src = yi0[:, :, ds(bi * NBLK, NBLK)]
            dst = yB[:, 0, ds(2 * bi, 2), :]
            if bi == 0:
                nc.vector.tensor_scalar_add(dst, src, float(-ZP_Y))
            else:
                nc.scalar.activation(dst, src, Copy, bias=float(-ZP_Y))
        emit_x_convert(0, 0)
        emit_x_convert(0, 1)
        # Sub-pass 1 lags 4 k-steps behind sub-pass 0: the first steps
        # demand y tiles at half rate, matching the still-ramping DMA
        # completion stream; sub-1's tail runs when supply is done.
        # Downstream sub-passes wait on sub-0's eviction, so the lag
        # costs nothing at the back end.
        LAG = 4
        for q in range(Q + LAG):
            if q < Q:
                if q > 0:
                    emit_y(q)
                if q % 2 == 0:
                    emit_xq_dma(q // 2 + 1)
                # convert the x half needed two k-steps ahead, one per step
                emit_x_convert((q + 2) // 2, (q + 2) % 2)
                emit_sub_q(0, q, ps0)
            if q >= LAG:
                emit_sub_q(1, q - LAG, ps1)
        evict_sub(0, ps0)
        evict_sub(1, ps1)

        # Phase B: sub-passes 2..7 free-run from SBUF
        for s in range(2, nsubs):
            psums = alloc_psums(s)
            for q in range(Q):
                emit_sub_q(s, q, psums)
            evict_sub(s, psums)

    nc.compile()
    return nc


_NC_CACHE = None
LAST_RESULT = None  # BassKernelResults of the most recent run (for profiling)


def _ensure_ntff_hook():
    """concourse's trace path imports antenv.axon_hooks, which is absent
    from this container's antenv stub. Provide it (with the real libaxon
    ctypes hook when available) so tracing works -- or degrades cleanly."""
    import sys
    import types

    try:
        import antenv.axon_hooks  # noqa: F401

        return
    except ImportError:
        pass
    mod = types.ModuleType("antenv.axon_hooks")
    holder = [None]
    mod.set_axon_ntff_profile_hook = lambda h: holder.__setitem__(0, h)
    mod.get_axon_ntff_profile_hook = lambda: holder[0]
    sys.modules["antenv.axon_hooks"] = mod
    try:
        import antenv

        antenv.axon_hooks = mod
    except ImportError:
        pass
    try:
        from trn_agent_boot.trn_boot import _ntff_profile_via_ctypes

        mod.set_axon_ntff_profile_hook(
            _ntff_profile_via_ctypes("/opt/axon/libaxon_pjrt.so")
        )
    except Exception:
        pass  # no hook -> concourse logs a warning and skips tracing


def kernel(x, y):
    global _NC_CACHE, LAST_RESULT
    _ensure_ntff_hook()
    from concourse.bass_utils import run_bass_kernel_spmd

    x = np.asarray(x)
    y = np.asarray(y)
    assert x.shape == (M, K) and y.shape == (K, N), (x.shape, y.shape)
    x8 = x.astype(np.int8) if x.dtype != np.int8 else x
    y8 = y.astype(np.int8) if y.dtype != np.int8 else y

    m_loc = M // M_SH
    n_loc = N // N_SH
    if _NC_CACHE is None:
        _NC_CACHE = build_nc(m_loc, K, n_loc)
    nc = _NC_CACHE

    in_maps = []
    for c in range(N_CORES):
        mi, nj = divmod(c, N_SH)
        in_maps.append(
            {
                "xt": np.ascontiguousarray(x8[mi * m_loc : (mi + 1) * m_loc].T),
                "y": np.ascontiguousarray(y8[:, nj * n_loc : (nj + 1) * n_loc]),
            }
        )
    res = run_bass_kernel_spmd(nc, in_maps, core_ids=list(range(N_CORES)))
    LAST_RESULT = res
    full = np.empty((M, N), dtype=np.float32)
    for c in range(N_CORES):
        mi, nj = divmod(c, N_SH)
        full[mi * m_loc : (mi + 1) * m_loc, nj * n_loc : (nj + 1) * n_loc] = (
            np.asarray(res.results[c]["out"])
        )
    return full
